# revision 30
# baseline (speedup 1.0000x reference)
"""Longformer banded self-attention on 8 trn2 NeuronCores.

Sharding: sequence-parallel. Core c (c = 4*b + g) handles batch b, tokens
[g*1024, (g+1)*1024). Host ships each core its token span plus a 64-token
halo on each side (so no device-to-device exchange is needed), pre-transposed
to [E, tokens] so the contraction dim lands on SBUF partitions.

Device pipeline per core:
  1. Q^T/K^T projections into [e_out, token] layout (lhsT = W tile, rhs = h^T),
     V into natural [token, e_out] layout augmented with a ones column per head
     (the ones column turns the P@V matmul into P@[V|1] which yields the
     softmax denominator for free). Projections run in float32r (full PE rate
     at N>=256, ~16x more accurate than bf16); results are evacuated to fp16.
  2. Banded attention per (128-query tile, 4-head group): scores computed
     TRANSPOSED St[key, query] via 2 matmuls [K=64, 128, 128] per head (key
     window = 256 = 2 blocks), exp on ScalarE with a constant -2 bias (pure
     overflow headroom; numerator and denominator scale identically), band
     mask applied as one fp16 tensor_tensor multiply against a
     host-precomputed per-tile mask (broadcast across heads via stride-0 AP
     dim; sequence edges baked into the mask data, SPMD-safe).
  3. P@[V|1] accumulated over the 2 key blocks in fp32 PSUM, rows normalized
     by the reciprocal of the ones-column sums, f32 rows DMAed out. bv is
     added on the host (a value bias passes through the softmax average
     exactly).

Scheduling: emission order = Tile priority. Input DMAs round-robin over the
three DMA-capable queues (SP/ACT/Pool); K^T/Q^T/V SBUF tensors are split
(3/2/9 tiles) and attention units are interleaved with the projection streams
in data-readiness order. Evacuations: K^T/Q^T on VectorE, V on ScalarE.
PSUM: psQ(2) + psV(1) + psS(2x2) + psPV(1) = 8 banks.

Measured (8-core SPMD, vs fp32 reference): rel err 4.5e-4; cost-model
per-core time ~49.6 us.
"""

import numpy as np
import ml_dtypes

import concourse.bass as bass
import concourse.bacc as bacc
import concourse.mybir as mybir
import concourse.tile as tile
from concourse.bass_utils import run_bass_kernel_spmd

BF16 = ml_dtypes.bfloat16

B, S, E, H, W = 2, 4096, 512, 8, 64
D = E // H            # 64
NCORES = 8
GROUPS = 4            # token groups per batch
SPAN = S // GROUPS    # 1024 tokens per core
HALO = 128            # halo tokens total (64 each side)
SPANH = SPAN + HALO   # 1152
NT = SPAN // 128      # 8 query tiles per core
KT = E // 128         # 4 contraction tiles
VA = H * (D + 1)      # 520: V augmented with ones column per head

_CACHE = {}


def build_nc():
    dt = mybir.dt
    nc = bacc.Bacc()

    hT_d = nc.dram_tensor("hT", [E, SPANH], dt.float32r, kind="ExternalInput")
    wq_d = nc.dram_tensor("wq", [E, E], dt.float32r, kind="ExternalInput")
    wk_d = nc.dram_tensor("wk", [E, E], dt.float32r, kind="ExternalInput")
    wv_d = nc.dram_tensor("wv", [E, VA], dt.float32r, kind="ExternalInput")
    bqc_d = nc.dram_tensor("bqc", [128, KT], dt.float32, kind="ExternalInput")
    bkc_d = nc.dram_tensor("bkc", [128, KT], dt.float32, kind="ExternalInput")
    m01_d = nc.dram_tensor("m01", [128, NT * 256], dt.float16,
                           kind="ExternalInput")
    out_d = nc.dram_tensor("out", [SPAN, E], dt.float32, kind="ExternalOutput")

    with tile.TileContext(nc) as tc:
        with tc.tile_pool(name="const", bufs=1) as const:
            bqc_sb = const.tile([128, KT], dt.float32, tag="bqc")
            bkc_sb = const.tile([128, KT], dt.float32, tag="bkc")
            m01_sb = const.tile([128, NT * 256], dt.float16, tag="m01")
            # spread DMA issue across the three DMA-capable queues
            # (SP, Activation, gpsimd)
            # round-robin the big input DMAs over the three DMA-capable
            # queues (SP, ACT, Pool), K/h first (Kt projections start first)
            hT_k, wq_k, wk_k, wv_k = [], [], [], []
            for k in range(KT):
                hT_k.append(const.tile([128, SPANH], dt.float32r,
                                       tag=f"hT{k}", name=f"hk{k}"))
                wq_k.append(const.tile([128, E], dt.float32r,
                                       tag=f"wq{k}", name=f"qk{k}"))
                wk_k.append(const.tile([128, E], dt.float32r,
                                       tag=f"wk{k}", name=f"kk{k}"))
                wv_k.append(const.tile([128, VA], dt.float32r,
                                       tag=f"wv{k}", name=f"vk{k}"))
            # hand-placed queues: wk first (small, gates every Kt matmul),
            # then hT; ACT's queue starts ~1.3us late (activation table load)
            def _sl(td, k):
                return td[k * 128:(k + 1) * 128, :]
            for q, xfers in (
                (nc.sync, [(wk_k[1], _sl(wk_d, 1)), (hT_k[0], _sl(hT_d, 0)),
                           (hT_k[3], _sl(hT_d, 3)), (wq_k[0], _sl(wq_d, 0)),
                           (wq_k[1], _sl(wq_d, 1)), (wv_k[1], _sl(wv_d, 1)),
                           (bkc_sb, bkc_d[:])]),
                (nc.scalar, [(wk_k[2], _sl(wk_d, 2)), (hT_k[1], _sl(hT_d, 1)),
                             (wq_k[2], _sl(wq_d, 2)), (wv_k[2], _sl(wv_d, 2)),
                             (bqc_sb, bqc_d[:])]),
                (nc.gpsimd, [(wk_k[0], _sl(wk_d, 0)), (wk_k[3], _sl(wk_d, 3)),
                             (hT_k[2], _sl(hT_d, 2)), (wq_k[3], _sl(wq_d, 3)),
                             (wv_k[0], _sl(wv_d, 0)), (wv_k[3], _sl(wv_d, 3))]),
            ):
                for sb, dr in xfers:
                    q.dma_start(sb[:], dr)
            nc.gpsimd.dma_start(m01_sb[:], m01_d[:])
            nbias_sb = const.tile([128, 1], dt.float32, tag="nbias")
            nc.gpsimd.memset(nbias_sb[:], -2.0)

            # PE warmup: the HAM clock gate needs ~3.4us of sustained PE
            # activity to reach 2.4GHz; the PE is otherwise idle during the
            # input-DMA window, so ramp it on dummy matmuls (results unread)
            warm_sb = const.tile([128, 512], dt.float16, tag="warm")
            nc.vector.memset(warm_sb[:], 0.0)

            # split result tensors for fine-grained attention deps
            # kt chunks: keys [0,512), [512,1024), [1024,1152); etile j at j*cw
            kt_ch = [const.tile([128, KT * 512], dt.float16, tag="kta", name="kta"),
                     const.tile([128, KT * 512], dt.float16, tag="ktb", name="ktb"),
                     const.tile([128, KT * 256], dt.float16, tag="ktc", name="ktc")]
            qt_h = [const.tile([128, KT * 512], dt.float16, tag="qt0", name="qt0"),
                    const.tile([128, KT * 512], dt.float16, tag="qt1", name="qt1")]
            v_t = [const.tile([128, VA], dt.float16, tag=f"v{t}", name=f"v{t}")
                   for t in range(9)]

            # ---------------- projections ----------------
            with tc.tile_pool(name="psQ", bufs=2, space=bass.MemorySpace.PSUM) as psQ, \
                 tc.tile_pool(name="probs", bufs=2) as probsp, \
                 tc.tile_pool(name="masked", bufs=2) as maskedp, \
                 tc.tile_pool(name="osb", bufs=2) as osbp, \
                 tc.tile_pool(name="rec", bufs=2) as recp:
                def warmup(psQ):
                    for w in range(6):
                        ps = psQ.tile([128, 512], dt.float32, tag="ps",
                                      name="pswarm")
                        nc.tensor.matmul(ps[:], warm_sb[:, 0:128], warm_sb[:],
                                         start=True, stop=True)

                def proj_k(ci, cw, j):
                    off = 896 if ci == 2 else ci * 512
                    ps = psQ.tile([128, 512], dt.float32, tag="ps", name="psk")
                    for k in range(KT):
                        nc.tensor.matmul(
                            ps[:, :cw],
                            wk_k[k][:, j * 128:(j + 1) * 128],
                            hT_k[k][:, off: off + cw],
                            start=(k == 0), stop=(k == KT - 1))
                    nc.vector.tensor_scalar_add(
                        kt_ch[ci][:, j * cw:(j + 1) * cw],
                        ps[:, :cw], bkc_sb[:, j:j + 1])

                def proj_q(c, j):
                    ps = psQ.tile([128, 512], dt.float32, tag="ps", name="psq")
                    for k in range(KT):
                        nc.tensor.matmul(
                            ps[:],
                            wq_k[k][:, j * 128:(j + 1) * 128],
                            hT_k[k][:, 64 + c * 512: 64 + (c + 1) * 512],
                            start=(k == 0), stop=(k == KT - 1))
                    if False:
                        nc.scalar.activation(
                            qt_h[c][:, j * 512:(j + 1) * 512], ps[:],
                            mybir.ActivationFunctionType.Identity,
                            bias=bqc_sb[:, j:j + 1])
                    else:
                        nc.vector.tensor_scalar_add(
                            qt_h[c][:, j * 512:(j + 1) * 512],
                            ps[:], bqc_sb[:, j:j + 1])

                def proj_v(psV, t):
                    # V_aug per 128-token tile (offset -64), evac on ACT;
                    # the per-head ones columns are memset directly (bv is
                    # folded into the output on the host)
                    for half in range(2):
                        ps = psV.tile([128, 512], dt.float32, tag="psv", name="psv")
                        for k in range(KT):
                            nc.tensor.matmul(
                                ps[:, 0:260],
                                hT_k[k][:, t * 128:(t + 1) * 128],
                                wv_k[k][:, half * 260:(half + 1) * 260],
                                start=(k == 0), stop=(k == KT - 1))
                        nc.scalar.copy(
                            v_t[t][:, half * 260:(half + 1) * 260], ps[:, 0:260])
                    nc.gpsimd.memset(
                        v_t[t][:].rearrange("p (a b) -> p a b", b=65)[:, :, 64:65],
                        1.0)

                def attn(psS, psPV, t):
                    osb = osbp.tile([128, 512], dt.float32, tag="osb")
                    for hg in range(2):
                        # scores^T [key, query]; local head i -> slot s(i)
                        # pairs (0,1),(2,3) must hit different PSUM banks
                        ps_s = psS.tile([128, 1024], dt.float32, tag="scores")
                        # blk-major so head pairs (rows 0-63 / 64-127 of the
                        # PE array, different PSUM banks) are issued
                        # back-to-back -> row-group concurrency on silicon
                        for blk in range(2):
                            ko = t * 128 + blk * 128
                            if ko >= 1024:
                                ci, cko, cw = 2, ko - 896, 256
                            else:
                                ci, cko, cw = ko // 512, ko % 512, 512
                            for i in range(4):
                                h = hg * 4 + i
                                j, sub = h // 2, h % 2
                                pr = 64 * sub
                                slot = (i % 2) * 2 + i // 2
                                nc.tensor.matmul(
                                    ps_s[:, slot * 256 + blk * 128:
                                         slot * 256 + (blk + 1) * 128],
                                    kt_ch[ci][pr:pr + 64,
                                              j * cw + cko: j * cw + cko + 128],
                                    qt_h[t // 4][pr:pr + 64,
                                                 j * 512 + (t % 4) * 128:
                                                 j * 512 + (t % 4 + 1) * 128],
                                    start=True, stop=True)
                        probs = probsp.tile([128, 1024], dt.float16, tag="probs")
                        # constant bias: exp(s-2) scales numerator and
                        # denominator identically (overflow headroom for fp16)
                        nc.scalar.activation(
                            probs[:], ps_s[:], mybir.ActivationFunctionType.Exp,
                            bias=nbias_sb[:])
                        masked = maskedp.tile([128, 1024], dt.float16, tag="masked")
                        nc.vector.tensor_mul(
                            masked[:].rearrange("p (s b x) -> p s b x", s=4, b=2),
                            probs[:].rearrange("p (s b x) -> p s b x", s=4, b=2),
                            m01_sb[:, t * 256:(t + 1) * 256].rearrange(
                                "p (b x) -> p b x", b=2)[:, None, :, :].broadcast_to(
                                    [128, 4, 2, 128]))
                        # P @ [V | 1]: local head i at psum col 65i
                        ps_pv = psPV.tile([128, 512], dt.float32, tag="pv")
                        for i in range(4):
                            h = hg * 4 + i
                            slot = (i % 2) * 2 + i // 2
                            for blk in range(2):
                                nc.tensor.matmul(
                                    ps_pv[:, i * 65:(i + 1) * 65],
                                    masked[:, slot * 256 + blk * 128:
                                           slot * 256 + (blk + 1) * 128],
                                    v_t[t + blk][:, h * 65:(h + 1) * 65],
                                    start=(blk == 0), stop=(blk == 1))
                        rec = recp.tile([128, 4], dt.float32, tag="rec")
                        nc.vector.reciprocal(
                            rec[:].unsqueeze(2),
                            ps_pv[:, 64:64 + 4 * 65].rearrange(
                                "p (c b) -> p c b", c=4)[:, :, 0:1])
                        nc.vector.tensor_mul(
                            osb[:, hg * 256:(hg + 1) * 256].rearrange(
                                "p (c b) -> p c b", c=4),
                            ps_pv[:, 0:4 * 65].rearrange(
                                "p (c b) -> p c b", c=4)[:, :, 0:64],
                            rec[:].unsqueeze(2).broadcast_to([128, 4, 64]))
                    nc.gpsimd.dma_start(out_d[t * 128:(t + 1) * 128, :], osb[:])

                # v2-interleave: stagger projections and attention units in
                # data-readiness order.
                with tc.tile_pool(name="psV", bufs=1,
                                  space=bass.MemorySpace.PSUM) as psV, \
                     tc.tile_pool(name="psS", bufs=2,
                                  space=bass.MemorySpace.PSUM) as psS, \
                     tc.tile_pool(name="psPV", bufs=1,
                                  space=bass.MemorySpace.PSUM) as psPV:
                    warmup(psQ)
                    for j in range(KT):
                        proj_k(0, 512, j)
                        proj_q(0, j)
                    proj_v(psV, 0); proj_v(psV, 1); proj_v(psV, 2)
                    proj_v(psV, 3)
                    attn(psS, psPV, 0)
                    attn(psS, psPV, 1)
                    for j in range(KT):
                        proj_k(1, 512, j)
                        proj_q(1, j)
                    attn(psS, psPV, 2)
                    proj_v(psV, 4); proj_v(psV, 5)
                    attn(psS, psPV, 3)
                    proj_v(psV, 6)
                    attn(psS, psPV, 4)
                    proj_v(psV, 7)
                    attn(psS, psPV, 5)
                    for j in range(2):
                        proj_k(2, 256, j)
                    proj_v(psV, 8)
                    for j in range(2, KT):
                        proj_k(2, 256, j)
                    attn(psS, psPV, 6)
                    attn(psS, psPV, 7)
    nc.finalize()
    return nc


def get_nc():
    if "nc" not in _CACHE:
        _CACHE["nc"] = build_nc()
    return _CACHE["nc"]


def make_in_maps(hidden_states, Wq, bq, Wk, bk, Wv, bv):
    hs = np.asarray(hidden_states, dtype=np.float32)
    Wq = np.asarray(Wq, dtype=np.float32)
    Wk = np.asarray(Wk, dtype=np.float32)
    Wv = np.asarray(Wv, dtype=np.float32)
    bq = np.asarray(bq, dtype=np.float32)
    bk = np.asarray(bk, dtype=np.float32)
    bv = np.asarray(bv, dtype=np.float32)

    scale = 1.0 / np.sqrt(D)
    wq_b = (Wq * scale).astype(np.float32)
    wk_b = Wk.astype(np.float32)
    wv_aug = np.zeros((E, VA), dtype=np.float32)
    for h in range(H):
        wv_aug[:, h * 65: h * 65 + 64] = Wv[:, h * 64:(h + 1) * 64]
    wv_b = wv_aug.astype(np.float32)

    bqc = ((bq * scale).reshape(KT, 128).T).astype(np.float32).copy()
    bkc = (bk.reshape(KT, 128).T).astype(np.float32).copy()

    y = np.arange(128)[:, None]
    x = np.arange(128)[None, :]
    m0_base = (x <= y).astype(np.float32)   # block0: prefix in x
    m1_base = (x >= y).astype(np.float32)   # block1: suffix in x

    in_maps = []
    for c in range(NCORES):
        b, g = c // GROUPS, c % GROUPS
        a0 = g * SPAN
        lo, hi = a0 - 64, a0 + SPAN + 64
        s0, s1 = max(lo, 0), min(hi, S)
        hT = np.zeros((E, SPANH), dtype=np.float32)
        hT[:, s0 - lo: s1 - lo] = np.ascontiguousarray(hs[b, s0:s1, :].T)
        m01 = np.zeros((128, NT * 256), dtype=np.float32)
        for t in range(NT):
            T = g * NT + t
            m0 = m0_base.copy()
            m1 = m1_base.copy()
            if T == 0:
                m0[y[:, 0] < 64, :] = 0.0    # keys before sequence start
            if T == (S // 128) - 1:
                m1[y[:, 0] >= 64, :] = 0.0   # keys past sequence end
            m01[:, t * 256: t * 256 + 128] = m0
            m01[:, t * 256 + 128: (t + 1) * 256] = m1
        in_maps.append({
            "hT": hT, "wq": wq_b, "wk": wk_b, "wv": wv_b,
            "bqc": bqc, "bkc": bkc,
            "m01": m01.astype(np.float16),
        })
    return in_maps


def run(in_maps, **kw):
    nc = get_nc()
    return run_bass_kernel_spmd(nc, in_maps, list(range(NCORES)), **kw)


def kernel(hidden_states, key, value, attention_mask, Wq, bq, Wk, bk, Wv, bv):
    in_maps = make_in_maps(hidden_states, Wq, bq, Wk, bk, Wv, bv)
    res = run(in_maps)
    out = np.stack([r["out"] for r in res.results])  # [8, 1024, 512]
    out = out.reshape(B, S, E).astype(np.float32)
    bv = np.asarray(bv, dtype=np.float32)
    if np.any(bv):
        out = out + bv[None, None, :]
    return out

